# revision 18
# baseline (speedup 1.0000x reference)
"""Cross-attention kernel for 8 trn2 NeuronCores.

Problem: B=2, Lq=Lk=2048, D=1024, H=16, dh=64.
  q/k/v = Linear(x); q,k L2-normalized per head; S = q@k.T * 1/8;
  key-pad mask -> -1e9; softmax; mask-aware renorm; eps-smooth toward
  uniform-over-valid; out = attn@v merged -> out_proj.

Sharding: core c handles batch b=c//4, heads [4*(c%4), 4*(c%4)+4)
(two "head pairs" hp of 2 heads each). Each core computes a partial
output-projection over its 256 head dims; the host sums the 8 partials
(4 per batch) and adds a combined bias.

Key optimizations vs the naive formulation:
  - Key compaction: masked keys contribute exactly 0 after the
    reference's mask+renorm, so the host gathers only the valid keys
    (padded to a 128 multiple). Halves k/v projection, S, exp, AV work.
  - exp(SCALE*S - 30000) == 0 exactly for the pad keys in the last
    tile(s) (per-partition bias in the transposed S layout).
  - Softmax denominator comes free from the AV matmul: lhsT is
    [v_head | ones] (head A) / [ones | v_head] (head B), so the same
    instruction yields O on one 64-partition half and the replicated
    rowsum on the other. No separate rowsum matmuls.
  - 0.9 eps-smoothing factor is folded into Wo on the host; the
    0.1*uniform term is token-independent after out_proj and is folded
    into a host-side per-batch constant.
  - q/k projections run as fp8e4 DoubleRow matmuls (2 contraction
    tiles per instruction, 2x PE throughput). The fp8 quantization
    error washes out through L2-norm + softmax (<2e-4 on the output).

Device layouts (partition dim first):
  xT     [d_in chunk, tokens]      (host pre-transposes inputs)
  qT/kT  [128 = 2 heads x 64, tokens]
  v2     [tokens, kt, 192] = [vA(64) | ones(64) | vB(64)] per kt
  S_T    [k-tile=128, q]           exp bias = per-partition pad mask
  O      [128, q] = [O_A | rs_A] and [rs_B | O_B] per head pair
"""

import ml_dtypes
import numpy as np

from concourse import bacc
import concourse.mybir as mybir
import concourse.tile as tile
from concourse.bass_utils import run_bass_kernel_spmd

F32 = mybir.dt.float32
BF16 = mybir.dt.bfloat16
FP8 = mybir.dt.float8e4
AF = mybir.ActivationFunctionType
DR = mybir.MatmulPerfMode.DoubleRow

B, L, D = 2, 2048, 1024
H, DH = 16, 64
HEADS_PER_CORE = 4          # -> 256 dims per core, 2 head-pairs
HPC = HEADS_PER_CORE * DH   # 256
SCALE = 0.125               # 1/sqrt(64) / ATTN_TEMP
EPS_SMOOTH = 0.1
MASK_BIAS = -30000.0
N_CORES = 8
QC = L // 512               # 4 q chunks
NCH = D // 128              # 8 contraction chunks for projections


def _chunks(total, width):
    off = 0
    out = []
    while off < total:
        w = min(width, total - off)
        out.append((off, w))
        off += w
    return out


def _build_nc(kt_k, kt_full):
    KP = kt_k * 128
    nc = bacc.Bacc(None)

    xqT = nc.dram_tensor("xqT", [D, L], FP8, kind="ExternalInput")
    xkT = nc.dram_tensor("xkT", [D, KP], FP8, kind="ExternalInput")
    xvT = nc.dram_tensor("xvT", [D, KP], BF16, kind="ExternalInput")
    wq8 = nc.dram_tensor("wq8", [D, HPC], FP8, kind="ExternalInput")
    wk8 = nc.dram_tensor("wk8", [D, HPC], FP8, kind="ExternalInput")
    wv_t = nc.dram_tensor("wv_t", [D, HPC], BF16, kind="ExternalInput")
    wo_t = nc.dram_tensor("wo_t", [HPC, D], BF16, kind="ExternalInput")
    bq = nc.dram_tensor("bq", [2, 1, 128], BF16, kind="ExternalInput")
    bk = nc.dram_tensor("bk", [2, 1, 128], BF16, kind="ExternalInput")
    bv = nc.dram_tensor("bv", [1, HPC], BF16, kind="ExternalInput")
    mbias = nc.dram_tensor("mbias", [128, kt_k], F32, kind="ExternalInput")
    partial = nc.dram_tensor("partial", [L, D], BF16, kind="ExternalOutput")

    with tile.TileContext(nc) as tc:
        with (
            tc.tile_pool(name="consts", bufs=1) as consts,
            tc.tile_pool(name="wpool", bufs=1) as wpool,
            tc.tile_pool(name="persist", bufs=1) as persist,
            tc.tile_pool(name="xstream", bufs=4) as xstream,
            tc.tile_pool(name="xvstream", bufs=5) as xvstream,
            tc.tile_pool(name="l2pool", bufs=4) as l2pool,
            tc.tile_pool(name="ppool", bufs=4) as ppool,
            tc.tile_pool(name="dpool", bufs=2) as dpool,
            tc.tile_pool(name="tpool", bufs=3) as tpool,
            tc.tile_pool(name="opool", bufs=6) as opool,
        ):
            # ---- constants ----
            ones_row = consts.tile([1, 512], BF16, tag="ones_row")
            nc.vector.memset(ones_row, 1.0)
            blockdiag = consts.tile([128, 128], BF16, tag="blockdiag")
            nc.vector.memset(blockdiag, 0.0)
            nc.vector.memset(blockdiag[0:64, 0:64], 1.0)
            nc.vector.memset(blockdiag[64:128, 64:128], 1.0)
            mbias_sb = consts.tile([128, kt_k], F32, tag="mbias")
            bias_sb = {}
            for name, hnd in (("q", bq), ("k", bk)):
                for hp in range(2):
                    t = consts.tile([1, 128], BF16, tag=f"b{name}{hp}")
                    bias_sb[(name, hp)] = t
            bv_sb = consts.tile([1, HPC], BF16, tag="bv")

            # ---- weights ----
            # q/k: fp8 DoubleRow layout [128, chunk-pair, 2, 256]
            w8 = {}
            for name, hnd in (("q", wq8), ("k", wk8)):
                t = wpool.tile([128, NCH // 2, 2, HPC], FP8, tag=f"w8{name}",
                               name=f"w8{name}")
                w8[name] = t
            nc.sync.dma_start(
                out=w8["k"],
                in_=wk8.rearrange("(c j p) m -> p c j m", p=128, j=2))
            for name, hnd in (("q", bq), ("k", bk)):
                for hp in range(2):
                    nc.sync.dma_start(out=bias_sb[(name, hp)], in_=hnd[hp])
            nc.sync.dma_start(out=mbias_sb, in_=mbias[:, :])
            nc.sync.dma_start(out=bv_sb, in_=bv[:, :])
            wv_sb = wpool.tile([128, NCH, HPC], BF16, tag="wv")
            wo_sb = wpool.tile([128, 2, D], BF16, tag="wo")

            # ---- persistent activations ----
            qTn = [persist.tile([128, L], BF16, tag=f"qTn{hp}", name=f"qTn{hp}")
                   for hp in range(2)]
            kTn = [persist.tile([128, KP], BF16, tag=f"kTn{hp}", name=f"kTn{hp}")
                   for hp in range(2)]
            # [vA | ones | ... | vB] per k-tile; ones come from the memset
            v2 = [persist.tile([128, kt_k, 192], BF16, tag=f"v2{hp}",
                               name=f"v2{hp}") for hp in range(2)]
            for hp in range(2):
                nc.gpsimd.memset(v2[hp], 1.0)
            ofin = [persist.tile([128, L], BF16, tag=f"ofin{hp}", name=f"ofin{hp}")
                    for hp in range(2)]

            # ---- phase A: projections ----
            with (
                tc.tile_pool(name="ps_proj", bufs=4, space="PSUM") as ps_proj,
                tc.tile_pool(name="ps_n2", bufs=2, space="PSUM") as ps_n2,
                tc.tile_pool(name="ps_vp", bufs=2, space="PSUM") as ps_vp,
            ):
                def qk_proj(name, xhnd, dst, chunks):
                    for off, W in chunks:
                        psums = [
                            ps_proj.tile([128, 512], F32, tag="proj",
                                         name=f"proj{i}")
                            for i in range(2)
                        ]
                        xt = xstream.tile([128, NCH // 2, 2, 512], FP8,
                                          tag="xt")
                        nc.sync.dma_start(
                            out=xt[:, :, :, 0:W],
                            in_=xhnd[:, off:off + W].rearrange(
                                "(c j p) n -> p c j n", p=128, j=2),
                        )
                        for c in range(NCH // 2):
                            for hp in range(2):
                                nc.tensor.matmul(
                                    psums[hp][:, 0:W],
                                    lhsT=w8[name][:, c, :,
                                                  hp * 128:(hp + 1) * 128],
                                    rhs=xt[:, c, :, 0:W],
                                    start=(c == 0),
                                    stop=False,
                                    perf_mode=DR,
                                )
                        for hp in range(2):
                            # + bias (broadcast along tokens via K=1 matmul)
                            nc.tensor.matmul(
                                psums[hp][:, 0:W],
                                lhsT=bias_sb[(name, hp)],
                                rhs=ones_row[:, 0:W],
                                start=False,
                                stop=True,
                            )
                            # L2 norm over each head's 64 dims
                            sq = l2pool.tile([128, 512], BF16, tag="sq")
                            nc.scalar.square(sq[:, 0:W], psums[hp][:, 0:W])
                            n2 = ps_n2.tile([128, 512], F32, tag="n2")
                            nc.tensor.matmul(
                                n2[:, 0:W], lhsT=blockdiag, rhs=sq[:, 0:W],
                                start=True, stop=True,
                            )
                            nlen = l2pool.tile([128, 512], F32, tag="nlen")
                            nc.scalar.activation(nlen[:, 0:W], n2[:, 0:W],
                                                 AF.Sqrt)
                            rnorm = l2pool.tile([128, 512], F32, tag="rnorm")
                            nc.vector.reciprocal_approx_fast(rnorm[:, 0:W],
                                                             nlen[:, 0:W])
                            nc.vector.tensor_mul(
                                dst[hp][:, off:off + W],
                                psums[hp][:, 0:W], rnorm[:, 0:W],
                            )

                # wv load early (v-proj interleaves with k-proj)
                nc.sync.dma_start(
                    out=wv_sb, in_=wv_t.rearrange("(c p) m -> p c m", p=128))

                def emit_v_pair(tp):
                    tts = [tp] + ([tp + 1] if tp + 1 < kt_k else [])
                    nt = len(tts)
                    vps = [ps_vp.tile([128, HPC], F32, tag="vp",
                                      name=f"vp{i}") for i in range(nt)]
                    xvt = xvstream.tile([128, NCH, 256], BF16, tag="xvt")
                    nc.sync.dma_start(
                        out=xvt[:, :, 0:128 * nt],
                        in_=xvT[:, tp * 128:tp * 128 + 128 * nt].rearrange(
                            "(c p) n -> p c n", p=128),
                    )
                    for c in range(NCH):
                        for i in range(nt):
                            nc.tensor.matmul(
                                vps[i],
                                lhsT=xvt[:, c, i * 128:(i + 1) * 128],
                                rhs=wv_sb[:, c, :],
                                start=(c == 0), stop=False,
                            )
                    for i, tt in enumerate(tts):
                        nc.tensor.matmul(
                            vps[i], lhsT=ones_row[:, 0:128], rhs=bv_sb,
                            start=False, stop=True,
                        )
                        for hp in range(2):
                            nc.vector.tensor_copy(
                                v2[hp][:, tt, 0:64],
                                vps[i][:, hp * 128:hp * 128 + 64])
                            nc.vector.tensor_copy(
                                v2[hp][:, tt, 128:192],
                                vps[i][:, hp * 128 + 64:hp * 128 + 128])

                # interleave k-chunks and v-pairs to hide DMA latency
                kchunks = _chunks(KP, 512)
                vpairs = list(range(0, kt_k, 2))
                for j in range(max(len(kchunks), len(vpairs))):
                    if j < len(kchunks):
                        qk_proj("k", xkT, kTn, [kchunks[j]])
                    if j < len(vpairs):
                        emit_v_pair(vpairs[j])

                # deferred weight loads (off the k-proj critical path)
                nc.sync.dma_start(
                    out=w8["q"],
                    in_=wq8.rearrange("(c j p) m -> p c j m", p=128, j=2))

                qk_proj("q", xqT, qTn, _chunks(L, 512))

                # wo needed only in phase C; emit last so it never contends
                nc.sync.dma_start(
                    out=wo_sb, in_=wo_t.rearrange("(h p) m -> p h m", p=128))

            # ---- phase B: attention ----
            with (
                tc.tile_pool(name="ps_S", bufs=2, space="PSUM") as ps_S,
                tc.tile_pool(name="ps_O", bufs=4, space="PSUM") as ps_O,
            ):
                # exact 2nd-order Taylor of exp (|logit|<=1/8):
                # t = a*s + b, p = t^2 + 0.5 with a=SCALE/sqrt(2), b=1/sqrt(2)
                PA = SCALE / np.sqrt(2.0)
                PB = 1.0 / np.sqrt(2.0)
                poly_kts = set()  # poly offload disabled (queue coupling)

                def emit_S(hp, qsl, kt):
                    ksl = slice(kt * 128, (kt + 1) * 128)
                    s_ps = ps_S.tile([128, 1024], F32, tag="s")
                    # S_T = k̂.T q̂ per head, row-packed (K=64 each)
                    nc.tensor.matmul(
                        s_ps[:, 0:512],
                        lhsT=kTn[hp][0:64, ksl],
                        rhs=qTn[hp][0:64, qsl],
                        start=True, stop=True,
                    )
                    nc.tensor.matmul(
                        s_ps[:, 512:1024],
                        lhsT=kTn[hp][64:128, ksl],
                        rhs=qTn[hp][64:128, qsl],
                        start=True, stop=True,
                    )
                    return s_ps

                def emit_P(s_ps, kt):
                    # P = exp(SCALE*S + pad_bias); pad keys -> 0
                    p_sb = ppool.tile([128, 1024], BF16, tag="p")
                    if kt in poly_kts:
                        # DVE Taylor path (all-valid tiles only)
                        t1 = tpool.tile([128, 1024], BF16, tag="t1")
                        nc.vector.tensor_scalar(
                            t1, s_ps, PA, PB,
                            mybir.AluOpType.mult, mybir.AluOpType.add)
                        t2 = tpool.tile([128, 1024], BF16, tag="t2")
                        nc.vector.tensor_mul(t2, t1, t1)
                        nc.vector.tensor_scalar_add(p_sb, t2, 0.5)
                    else:
                        nc.scalar.activation(
                            p_sb, s_ps, AF.Exp,
                            bias=mbias_sb[:, kt:kt + 1], scale=SCALE,
                        )
                    return p_sb

                def emit_AV(hp, oA, oB, p_sb, kt):
                    # O += [v|1].T @ P : O_A on parts 0:64 + rs_A
                    # replicated on 64:128 (head B mirrored)
                    nc.tensor.matmul(
                        oA,
                        lhsT=v2[hp][:, kt, 0:128],
                        rhs=p_sb[:, 0:512],
                        start=(kt == 0), stop=(kt == kt_k - 1),
                    )
                    nc.tensor.matmul(
                        oB,
                        lhsT=v2[hp][:, kt, 64:192],
                        rhs=p_sb[:, 512:1024],
                        start=(kt == 0), stop=(kt == kt_k - 1),
                    )

                def emit_div1(hp, qsl, oA, oB):
                    # division part 1: pack rs, reciprocal, start swap DMA
                    rspack = dpool.tile([128, 512], F32, tag="rspack")
                    nc.vector.tensor_copy(rspack[64:128, :], oA[64:128, :])
                    nc.vector.tensor_copy(rspack[0:64, :], oB[0:64, :])
                    rr = dpool.tile([128, 512], F32, tag="rr")
                    nc.vector.reciprocal_approx_fast(rr, rspack)
                    rsw = dpool.tile([128, 512], F32, tag="rsw")
                    nc.sync.dma_start(out=rsw[0:64, :], in_=rr[64:128, :])
                    nc.sync.dma_start(out=rsw[64:128, :], in_=rr[0:64, :])
                    return rsw

                def emit_div2(hp, qsl, oA, oB, rsw):
                    # division part 2: ofin = O / rs (0.9 folded into Wo)
                    nc.vector.tensor_mul(
                        ofin[hp][0:64, qsl], oA[0:64, :], rsw[0:64, :])
                    nc.vector.tensor_mul(
                        ofin[hp][64:128, qsl], oB[64:128, :], rsw[64:128, :])

                prev_div = None
                for qc in range(QC):
                    qsl = slice(qc * 512, (qc + 1) * 512)
                    for hp in range(2):
                        oA = ps_O.tile([128, 512], F32, tag="o", name="oA")
                        oB = ps_O.tile([128, 512], F32, tag="o", name="oB")
                        # software pipeline: S one k-tile ahead of P/AV;
                        # poly tiles' AV deferred one iter so the 3-op DVE
                        # chain never blocks the in-order PE queue; previous
                        # block's division emitted mid-stream so it never
                        # blocks ACT/PE queues
                        s_cur = emit_S(hp, qsl, 0)
                        pending = []
                        for kt in range(kt_k):
                            p_sb = emit_P(s_cur, kt)
                            if kt + 1 < kt_k:
                                s_cur = emit_S(hp, qsl, kt + 1)
                            if kt == 1 and prev_div is not None:
                                prev_rsw = emit_div1(*prev_div)
                            if kt == 4 and prev_div is not None:
                                emit_div2(*prev_div, prev_rsw)
                                prev_div = None
                            pending.append((p_sb, kt))
                            if kt not in poly_kts or kt == 0:
                                for pp, pk in pending:
                                    emit_AV(hp, oA, oB, pp, pk)
                                pending = []
                        for pp, pk in pending:
                            emit_AV(hp, oA, oB, pp, pk)
                        prev_div = ((hp, qsl, oA, oB))
                if prev_div is not None:
                    rsw = emit_div1(*prev_div)
                    emit_div2(*prev_div, rsw)

            # ---- phase C: partial output projection ----
            with tc.tile_pool(name="ps_out", bufs=6, space="PSUM") as ps_out:
                for tt in range(L // 128):
                    tsl = slice(tt * 128, (tt + 1) * 128)
                    for nh in range(2):
                        nsl = slice(nh * 512, (nh + 1) * 512)
                        op = ps_out.tile([128, 512], F32, tag="oproj")
                        nc.tensor.matmul(
                            op, lhsT=ofin[0][:, tsl], rhs=wo_sb[:, 0, nsl],
                            start=True, stop=False,
                        )
                        nc.tensor.matmul(
                            op, lhsT=ofin[1][:, tsl], rhs=wo_sb[:, 1, nsl],
                            start=False, stop=True,
                        )
                        ost = opool.tile([128, 512], BF16, tag="ost")
                        if (tt * 2 + nh) % 2 == 0:
                            nc.vector.tensor_copy(ost, op)
                        else:
                            nc.scalar.copy(ost, op)
                        nc.sync.dma_start(out=partial[tsl, nsl], in_=ost)

    nc.finalize()
    return nc



_NC_CACHE = {}


def _get_nc(kt_k, kt_full):
    key = (kt_k, kt_full)
    if key not in _NC_CACHE:
        _NC_CACHE[key] = _build_nc(kt_k, kt_full)
    return _NC_CACHE[key]


def kernel(q_in, k_in, v_in, kv_pad_mask, Wq, bq, Wk, bk, Wv, bv, Wo, bo,
           _trace=False):
    f32 = np.float32
    bf = ml_dtypes.bfloat16
    f8 = ml_dtypes.float8_e4m3fn
    q_in = np.asarray(q_in, f32)
    k_in = np.asarray(k_in, f32)
    v_in = np.asarray(v_in, f32)
    mask = np.asarray(kv_pad_mask, bool)
    Wq, bq, Wk, bk, Wv, bv, Wo, bo = (
        np.asarray(a, f32) for a in (Wq, bq, Wk, bk, Wv, bv, Wo, bo)
    )

    idx = [np.flatnonzero(~mask[b]) for b in range(B)]
    nv = [len(i) for i in idx]
    kt_k = max(1, max((n + 127) // 128 for n in nv))
    KP = kt_k * 128
    kt_full = min(n // 128 for n in nv)   # tiles < kt_full are all-valid
    nc = _get_nc(kt_k, kt_full)

    # per-batch host prep
    xq8, xk8, xvT, mb, hostconst = {}, {}, {}, {}, {}
    for b in range(B):
        xq8[b] = np.ascontiguousarray(q_in[b].T).astype(f8)
        kc = np.zeros((KP, D), f32)
        kc[:nv[b]] = k_in[b][idx[b]]
        xk8[b] = np.ascontiguousarray(kc.T).astype(f8)
        vc = np.zeros((KP, D), f32)
        vc[:nv[b]] = v_in[b][idx[b]]
        xvT[b] = np.ascontiguousarray(vc.T).astype(bf)
        mrow = np.where(np.arange(KP) < nv[b], 0.0, MASK_BIAS).astype(f32)
        mb[b] = np.ascontiguousarray(mrow.reshape(kt_k, 128).T)
        # 0.1 * uniform-over-valid term is token-independent after out_proj
        n = max(float(nv[b]), 1.0)
        valid = (~mask[b]).astype(f32) / n
        vmean_full = (valid @ v_in[b]) @ Wv.T + bv
        hostconst[b] = bo + EPS_SMOOTH * (vmean_full @ Wo.T)

    in_maps = []
    for core in range(N_CORES):
        b = core // 4
        h0 = (core % 4) * HEADS_PER_CORE
        rows = slice(h0 * DH, h0 * DH + HPC)
        in_maps.append({
            "xqT": xq8[b],
            "xkT": xk8[b],
            "xvT": xvT[b],
            "wq8": np.ascontiguousarray(Wq[rows].T).astype(f8),
            "wk8": np.ascontiguousarray(Wk[rows].T).astype(f8),
            "wv_t": np.ascontiguousarray(Wv[rows].T).astype(bf),
            "wo_t": np.ascontiguousarray(
                (1.0 - EPS_SMOOTH) * Wo[:, rows].T).astype(bf),
            "bq": np.ascontiguousarray(bq[rows].reshape(2, 1, 128)).astype(bf),
            "bk": np.ascontiguousarray(bk[rows].reshape(2, 1, 128)).astype(bf),
            "bv": np.ascontiguousarray(bv[rows].reshape(1, HPC)).astype(bf),
            "mbias": mb[b],
        })

    res = run_bass_kernel_spmd(nc, in_maps, core_ids=list(range(N_CORES)),
                               trace=_trace)
    out = np.zeros((B, L, D), f32)
    for core in range(N_CORES):
        out[core // 4] += res.results[core]["partial"].astype(f32)
    for b in range(B):
        out[b] += hostconst[b][None, :]
    if _trace:
        kernel._last_result = res
    return out
